# revision 1
# baseline (speedup 1.0000x reference)
"""Trainium2 Bass kernel for nn_MultiModalSplitNorm (static grouped GEMM / MoE).

Problem: x [16384, 4096] f32, W [4, 4096, 4096] bf16, group_sizes = [4096]*4.
Output: y[t] = x[t] @ W[g(t)].T  (bf16 matmul, f32 accumulate/output).

Sharding (8 cores): expert-parallel x output-column-parallel.
Core c handles expert g = c//2, output columns half h = c%2:
    y[g*4096:(g+1)*4096, h*2048:(h+1)*2048] =
        x[g*4096:(g+1)*4096] @ W[g, h*2048:(h+1)*2048, :].T

Per-core kernel (T=4096 tokens, K=4096 contraction, O=2048 outputs):
  - W half is loaded once, DMA-transposed DRAM->SBUF to [K, O] layout and
    kept resident (128 KB/partition).
  - x streams in 128-token blocks: DMA f32 -> DVE cast bf16 -> DMA-transpose
    SBUF->SBUF to [K, 128] tiles (lhsT).
  - PE: for each of 32 K-blocks, 4 matmuls (N=512) accumulate into 4 PSUM
    banks; banks double-buffered across token blocks.
  - PSUM -> SBUF copy (f32) -> DMA out.
"""

import os
import sys

import numpy as np

# ---- constants (hardcoded per spec; kernel.py must be self-contained) ----
NUM_EXPERTS = 4
GROUP = 4096  # tokens per expert
HIDDEN = 4096  # contraction dim
TOTAL = NUM_EXPERTS * GROUP
N_CORES = 8
O_HALF = HIDDEN // 2  # 2048 output columns per core

P = 128
IB = HIDDEN // P  # 32 k-blocks
NB = 512  # matmul moving free dim (one PSUM bank)
OB = O_HALF // NB  # 4 psum banks per token block
XCH = 2  # x row-block loaded in 2 chunks of 2048 cols
CHW = HIDDEN // XCH


def _ensure_paths():
    for p in ("/opt/trn_rl_repo", "/root/.axon_site", "/root/.axon_site/_ro/pypackages"):
        if os.path.isdir(p) and p not in sys.path:
            sys.path.append(p)
    try:
        import concourse  # noqa: F401
    except ImportError:
        raise RuntimeError("concourse not importable; check PYTHONPATH")


_NC_CACHE = {}


def build_nc(tb_count=GROUP // P):
    """Build + compile the per-core Bass program. tb_count = token blocks."""
    if tb_count in _NC_CACHE:
        return _NC_CACHE[tb_count]
    _ensure_paths()
    import concourse.mybir as mybir
    import concourse.tile as tile
    from concourse import bacc

    T = tb_count * P

    nc = bacc.Bacc("TRN2", target_bir_lowering=False, debug=False)
    x_d = nc.dram_tensor("x", [T, HIDDEN], mybir.dt.float32, kind="ExternalInput")
    w_d = nc.dram_tensor("w", [O_HALF, HIDDEN], mybir.dt.bfloat16, kind="ExternalInput")
    y_d = nc.dram_tensor("y", [T, O_HALF], mybir.dt.float32, kind="ExternalOutput")
    x_ap, w_ap, y_ap = x_d.ap(), w_d.ap(), y_d.ap()

    with tile.TileContext(nc) as tc:
        from contextlib import ExitStack

        with ExitStack() as ctx:
            wt_pool = ctx.enter_context(tc.tile_pool(name="wt", bufs=1))
            xf_pool = ctx.enter_context(tc.tile_pool(name="xf", bufs=2))
            xb_pool = ctx.enter_context(tc.tile_pool(name="xb", bufs=2))
            xt_pool = ctx.enter_context(tc.tile_pool(name="xt", bufs=2))
            out_pool = ctx.enter_context(tc.tile_pool(name="out", bufs=2))
            psum_pool = ctx.enter_context(
                tc.tile_pool(name="psum", bufs=2, space="PSUM")
            )

            # ---- W^T resident: 32 tiles [128, O_HALF] bf16 ----
            wT = []
            for ib in range(IB):
                t = wt_pool.tile(
                    [P, O_HALF], mybir.dt.bfloat16, name=f"wT{ib}", tag=f"wT{ib}"
                )
                # transpose-load W[:, ib*128:(ib+1)*128] -> [128, O_HALF]
                nc.sync.dma_start_transpose(t[:], w_ap[:, ib * P : (ib + 1) * P])
                wT.append(t)

            # ---- stream token blocks ----
            for tb in range(tb_count):
                r0 = tb * P
                xT = xt_pool.tile([P, IB, P], mybir.dt.bfloat16, name=f"xT_{tb}", tag="xT")
                for ch in range(XCH):
                    c0 = ch * CHW
                    xf = xf_pool.tile(
                        [P, CHW], mybir.dt.float32, name=f"xf_{tb}_{ch}", tag="xf"
                    )
                    nc.sync.dma_start(xf[:], x_ap[r0 : r0 + P, c0 : c0 + CHW])
                    xb = xb_pool.tile(
                        [P, CHW], mybir.dt.bfloat16, name=f"xb_{tb}_{ch}", tag="xb"
                    )
                    nc.vector.tensor_copy(xb[:], xf[:])
                    nc.sync.dma_start_transpose(
                        xT[:, ch * (IB // XCH) : (ch + 1) * (IB // XCH), :], xb[:]
                    )

                ps = [
                    psum_pool.tile(
                        [P, NB], mybir.dt.float32, name=f"ps_{tb}_{ob}", tag=f"ps{ob}"
                    )
                    for ob in range(OB)
                ]
                for ib in range(IB):
                    for ob in range(OB):
                        nc.tensor.matmul(
                            ps[ob][:],
                            xT[:, ib, :],
                            wT[ib][:, ob * NB : (ob + 1) * NB],
                            start=(ib == 0),
                            stop=(ib == IB - 1),
                        )

                yo = out_pool.tile(
                    [P, O_HALF], mybir.dt.float32, name=f"yo_{tb}", tag="yo"
                )
                for ob in range(OB):
                    nc.any.tensor_copy(out=yo[:, ob * NB : (ob + 1) * NB], in_=ps[ob][:])
                nc.sync.dma_start(y_ap[r0 : r0 + P, :], yo[:])

    nc.compile()
    _NC_CACHE[tb_count] = nc
    return nc


def _shard_inputs(x, W):
    import ml_dtypes

    x = np.asarray(x)
    if x.dtype != np.float32:
        x = x.astype(np.float32)
    W = np.asarray(W)
    if W.dtype != ml_dtypes.bfloat16:
        W = W.astype(ml_dtypes.bfloat16)
    in_maps = []
    for c in range(N_CORES):
        g, h = c // 2, c % 2
        in_maps.append(
            {
                "x": np.ascontiguousarray(x[g * GROUP : (g + 1) * GROUP]),
                "w": np.ascontiguousarray(W[g, h * O_HALF : (h + 1) * O_HALF, :]),
            }
        )
    return in_maps


def kernel(x, W, group_sizes=None, **_ignored):
    if group_sizes is not None:
        gs = np.asarray(group_sizes).astype(np.int64)
        assert gs.shape == (NUM_EXPERTS,) and np.all(gs == GROUP), (
            f"kernel compiled for static group_sizes=[{GROUP}]*{NUM_EXPERTS}, got {gs}"
        )
    _ensure_paths()
    from concourse.bass_utils import run_bass_kernel_spmd

    nc = build_nc()
    in_maps = _shard_inputs(x, W)
    res = run_bass_kernel_spmd(nc, in_maps, core_ids=list(range(N_CORES)))
    y = np.empty((TOTAL, HIDDEN), dtype=np.float32)
    for c in range(N_CORES):
        g, h = c // 2, c % 2
        y[g * GROUP : (g + 1) * GROUP, h * O_HALF : (h + 1) * O_HALF] = res.results[c][
            "y"
        ]
    return y


# revision 4
# speedup vs baseline: 1.0467x; 1.0467x over previous
"""Trainium2 Bass kernel for nn_MultiModalSplitNorm (static grouped GEMM / MoE).

Problem: x [16384, 4096] f32, W [4, 4096, 4096] bf16, group_sizes = [4096]*4.
Output: y[t] = x[t] @ W[g(t)].T  (bf16 matmul, f32 accumulate/output).

Sharding (8 cores): expert-parallel x output-column-parallel.
Core c handles expert g = c//2, output columns half h = c%2:
    y[g*4096:(g+1)*4096, h*2048:(h+1)*2048] =
        x[g*4096:(g+1)*4096] @ W[g, h*2048:(h+1)*2048, :].T

Per-core kernel (T=4096 tokens, K=4096 contraction, O=2048 outputs):
  - W half is loaded once, DMA-transposed DRAM->SBUF to [K, O] layout and
    kept resident (128 KB/partition).
  - x streams in 128-token blocks: DMA f32 -> DVE cast bf16 -> DMA-transpose
    SBUF->SBUF to [K, 128] tiles (lhsT).
  - PE: for each of 32 K-blocks, 4 matmuls (N=512) accumulate into 4 PSUM
    banks; banks double-buffered across token blocks.
  - PSUM -> SBUF copy (f32) -> DMA out.
"""

import os
import sys

import numpy as np

# ---- constants (hardcoded per spec; kernel.py must be self-contained) ----
NUM_EXPERTS = 4
GROUP = 4096  # tokens per expert
HIDDEN = 4096  # contraction dim
TOTAL = NUM_EXPERTS * GROUP
N_CORES = 8
O_HALF = HIDDEN // 2  # 2048 output columns per core

P = 128
IB = HIDDEN // P  # 32 k-blocks
NB = 512  # matmul moving free dim (one PSUM bank)
OB = O_HALF // NB  # 4 psum banks per token block
XCH = 2  # x row-block loaded in 2 chunks of 2048 cols
CHW = HIDDEN // XCH


def _ensure_paths():
    for p in ("/opt/trn_rl_repo", "/root/.axon_site", "/root/.axon_site/_ro/pypackages"):
        if os.path.isdir(p) and p not in sys.path:
            sys.path.append(p)
    try:
        import concourse  # noqa: F401
    except ImportError:
        raise RuntimeError("concourse not importable; check PYTHONPATH")


_NC_CACHE = {}


def build_nc(tb_count=GROUP // P):
    """Build + compile the per-core Bass program. tb_count = token blocks."""
    if tb_count in _NC_CACHE:
        return _NC_CACHE[tb_count]
    _ensure_paths()
    import concourse.mybir as mybir
    import concourse.tile as tile
    from concourse import bacc

    T = tb_count * P

    nc = bacc.Bacc("TRN2", target_bir_lowering=False, debug=False)
    x_d = nc.dram_tensor("x", [T, HIDDEN], mybir.dt.float32, kind="ExternalInput")
    # w is shipped pre-transposed from the host: [HIDDEN, O_HALF] = W_half.T
    # (the math consumes W.T; this is a sharding-time layout choice that lets
    # the weight stream in as full-bandwidth contiguous DMA)
    w_d = nc.dram_tensor("w", [HIDDEN, O_HALF], mybir.dt.bfloat16, kind="ExternalInput")
    y_d = nc.dram_tensor("y", [T, O_HALF], mybir.dt.float32, kind="ExternalOutput")
    x_ap, w_ap, y_ap = x_d.ap(), w_d.ap(), y_d.ap()

    with tile.TileContext(nc) as tc:
        from contextlib import ExitStack

        with ExitStack() as ctx:
            wt_pool = ctx.enter_context(tc.tile_pool(name="wt", bufs=1))
            xf_pool = ctx.enter_context(tc.tile_pool(name="xf", bufs=2))
            xb_pool = ctx.enter_context(tc.tile_pool(name="xb", bufs=2))
            xt_pool = ctx.enter_context(tc.tile_pool(name="xt", bufs=2))
            out_pool = ctx.enter_context(tc.tile_pool(name="out", bufs=2))
            psum_pool = ctx.enter_context(
                tc.tile_pool(name="psum", bufs=2, space="PSUM")
            )

            # ---- W^T resident: 32 tiles [128, O_HALF] bf16 ----
            # Loaded on the *scalar* HWDGE queue so the x-side pipeline on the
            # sync queue (loads/transposes/stores) is never stuck behind the
            # 16.8 MB weight bulk load; per-tile deps let matmuls start as
            # soon as wT[0] lands.
            wT = []
            for ib in range(IB):
                t = wt_pool.tile(
                    [P, O_HALF], mybir.dt.bfloat16, name=f"wT{ib}", tag=f"wT{ib}"
                )
                nc.scalar.dma_start(t[:], w_ap[ib * P : (ib + 1) * P, :])
                wT.append(t)

            # ---- stream token blocks ----
            for tb in range(tb_count):
                r0 = tb * P
                xT = xt_pool.tile([P, IB, P], mybir.dt.bfloat16, name=f"xT_{tb}", tag="xT")
                for ch in range(XCH):
                    c0 = ch * CHW
                    xf = xf_pool.tile(
                        [P, CHW], mybir.dt.float32, name=f"xf_{tb}_{ch}", tag="xf"
                    )
                    nc.sync.dma_start(xf[:], x_ap[r0 : r0 + P, c0 : c0 + CHW])
                    xb = xb_pool.tile(
                        [P, CHW], mybir.dt.bfloat16, name=f"xb_{tb}_{ch}", tag="xb"
                    )
                    nc.vector.tensor_copy(xb[:], xf[:])
                    nc.sync.dma_start_transpose(
                        xT[:, ch * (IB // XCH) : (ch + 1) * (IB // XCH), :], xb[:]
                    )

                ps = [
                    psum_pool.tile(
                        [P, NB], mybir.dt.float32, name=f"ps_{tb}_{ob}", tag=f"ps{ob}"
                    )
                    for ob in range(OB)
                ]
                for ib in range(IB):
                    for ob in range(OB):
                        nc.tensor.matmul(
                            ps[ob][:],
                            xT[:, ib, :],
                            wT[ib][:, ob * NB : (ob + 1) * NB],
                            start=(ib == 0),
                            stop=(ib == IB - 1),
                        )

                yo = out_pool.tile(
                    [P, O_HALF], mybir.dt.float32, name=f"yo_{tb}", tag="yo"
                )
                for ob in range(OB):
                    nc.any.tensor_copy(out=yo[:, ob * NB : (ob + 1) * NB], in_=ps[ob][:])
                nc.sync.dma_start(y_ap[r0 : r0 + P, :], yo[:])

    nc.compile()
    _NC_CACHE[tb_count] = nc
    return nc


def _shard_inputs(x, W):
    import ml_dtypes

    x = np.asarray(x)
    if x.dtype != np.float32:
        x = x.astype(np.float32)
    W = np.asarray(W)
    if W.dtype != ml_dtypes.bfloat16:
        W = W.astype(ml_dtypes.bfloat16)
    in_maps = []
    for c in range(N_CORES):
        g, h = c // 2, c % 2
        in_maps.append(
            {
                "x": np.ascontiguousarray(x[g * GROUP : (g + 1) * GROUP]),
                # ship the weight shard transposed: [HIDDEN, O_HALF]
                "w": np.ascontiguousarray(W[g, h * O_HALF : (h + 1) * O_HALF, :].T),
            }
        )
    return in_maps


def kernel(x, W, group_sizes=None, **_ignored):
    if group_sizes is not None:
        gs = np.asarray(group_sizes).astype(np.int64)
        assert gs.shape == (NUM_EXPERTS,) and np.all(gs == GROUP), (
            f"kernel compiled for static group_sizes=[{GROUP}]*{NUM_EXPERTS}, got {gs}"
        )
    _ensure_paths()
    from concourse.bass_utils import run_bass_kernel_spmd

    nc = build_nc()
    in_maps = _shard_inputs(x, W)
    res = run_bass_kernel_spmd(nc, in_maps, core_ids=list(range(N_CORES)))
    y = np.empty((TOTAL, HIDDEN), dtype=np.float32)
    for c in range(N_CORES):
        g, h = c // 2, c % 2
        y[g * GROUP : (g + 1) * GROUP, h * O_HALF : (h + 1) * O_HALF] = res.results[c][
            "y"
        ]
    return y


# revision 5
# speedup vs baseline: 1.0745x; 1.0266x over previous
"""Trainium2 Bass kernel for nn_MultiModalSplitNorm (static grouped GEMM / MoE).

Problem: x [16384, 4096] f32, W [4, 4096, 4096] bf16, group_sizes = [4096]*4.
Output: y[t] = x[t] @ W[g(t)].T  (bf16 matmul, f32 accumulate/output).

Sharding (8 cores): expert-parallel x output-column-parallel.
Core c handles expert g = c//2, output columns half h = c%2:
    y[g*4096:(g+1)*4096, h*2048:(h+1)*2048] =
        x[g*4096:(g+1)*4096] @ W[g, h*2048:(h+1)*2048, :].T

Host-side sharding ships both operands in the layout the PE consumes
(layout-only transforms; all arithmetic, including the bf16 cast of x,
happens on device):
  - w: [HIDDEN, O_HALF] = W_half.T           (contiguous weight stream)
  - x: [TB, HIDDEN, 128] t-block-tiled x.T   (one contiguous 2 MB read per
                                              128-token block)

Per-core kernel (T=4096 tokens, K=4096 contraction, O=2048 outputs):
  - W^T streamed once on the scalar HWDGE queue, resident in SBUF
    (128 KB/partition).
  - per token block: one 2 MB DMA (sync queue) -> DVE cast f32->bf16 ->
    lhsT tiles [128, 128].
  - PE: per K-block ldweights(x^T tile) + 4 matmuls (N=512) accumulating
    into 4 PSUM banks; even/odd token blocks use disjoint bank groups
    (double buffering). The first two token blocks are interleaved K-major
    so PE weight-tile consumption (1.7 us/tile) matches the W stream
    arrival rate (~1.5 us/tile) instead of stalling.
  - PSUM -> SBUF copy (f32) -> DMA out.

No DMA-transpose instructions anywhere: transpose<->copy transitions
serialize the whole DMA subsystem (HW hang workaround) and were measured
to throttle the weight stream ~2.4x during the prologue.
"""

import os
import sys

import numpy as np

# ---- constants (hardcoded per spec; kernel.py must be self-contained) ----
NUM_EXPERTS = 4
GROUP = 4096  # tokens per expert
HIDDEN = 4096  # contraction dim
TOTAL = NUM_EXPERTS * GROUP
N_CORES = 8
O_HALF = HIDDEN // 2  # 2048 output columns per core

P = 128
IB = HIDDEN // P  # 32 k-blocks
NB = 512  # matmul moving free dim (one PSUM bank)
OB = O_HALF // NB  # 4 psum banks per token block


def _ensure_paths():
    for p in ("/opt/trn_rl_repo", "/root/.axon_site", "/root/.axon_site/_ro/pypackages"):
        if os.path.isdir(p) and p not in sys.path:
            sys.path.append(p)
    try:
        import concourse  # noqa: F401
    except ImportError:
        raise RuntimeError("concourse not importable; check PYTHONPATH")


_NC_CACHE = {}


def build_nc(tb_count=GROUP // P):
    """Build + compile the per-core Bass program. tb_count = 128-token blocks."""
    if tb_count in _NC_CACHE:
        return _NC_CACHE[tb_count]
    _ensure_paths()
    import concourse.mybir as mybir
    import concourse.tile as tile
    from concourse import bacc

    nc = bacc.Bacc("TRN2", target_bir_lowering=False, debug=False)
    x_d = nc.dram_tensor(
        "x", [tb_count, HIDDEN, P], mybir.dt.float32, kind="ExternalInput"
    )
    w_d = nc.dram_tensor("w", [HIDDEN, O_HALF], mybir.dt.bfloat16, kind="ExternalInput")
    y_d = nc.dram_tensor("y", [tb_count * P, O_HALF], mybir.dt.float32, kind="ExternalOutput")
    x_ap, w_ap, y_ap = x_d.ap(), w_d.ap(), y_d.ap()

    with tile.TileContext(nc) as tc:
        from contextlib import ExitStack

        with ExitStack() as ctx:
            wt_pool = ctx.enter_context(tc.tile_pool(name="wt", bufs=1))
            xf_pool = ctx.enter_context(tc.tile_pool(name="xf", bufs=2))
            xb_pool = ctx.enter_context(tc.tile_pool(name="xb", bufs=2))
            out_pool = ctx.enter_context(tc.tile_pool(name="out", bufs=2))
            psum_pool = ctx.enter_context(
                tc.tile_pool(name="psum", bufs=1, space="PSUM")
            )

            # ---- W^T resident: 32 tiles [128, O_HALF] bf16, scalar queue ----
            wT = []
            for ib in range(IB):
                t = wt_pool.tile(
                    [P, O_HALF], mybir.dt.bfloat16, name=f"wT{ib}", tag=f"wT{ib}"
                )
                nc.scalar.dma_start(t[:], w_ap[ib * P : (ib + 1) * P, :])
                wT.append(t)

            def load_cast(tb):
                """DMA one token block and cast to bf16 lhsT tiles."""
                xf = xf_pool.tile(
                    [P, IB, P], mybir.dt.float32, name=f"xf_{tb}", tag="xf"
                )
                nc.sync.dma_start(xf[:], x_ap[tb].rearrange("(ib p) t -> p ib t", p=P))
                xb = xb_pool.tile(
                    [P, IB, P], mybir.dt.bfloat16, name=f"xb_{tb}", tag="xb"
                )
                h = IB // 2
                nc.vector.tensor_copy(xb[:, :h, :], xf[:, :h, :])
                nc.vector.tensor_copy(xb[:, h:, :], xf[:, h:, :])
                return xb

            def alloc_psum(tb):
                grp = (tb % 2) * OB  # even tb -> banks 0-3, odd tb -> banks 4-7
                return [
                    psum_pool.tile(
                        [P, NB],
                        mybir.dt.float32,
                        name=f"ps_{tb}_{ob}",
                        tag=f"bank{grp + ob}",
                    )
                    for ob in range(OB)
                ]

            def mm_group(ps, xb, ib):
                for ob in range(OB):
                    nc.tensor.matmul(
                        ps[ob][:],
                        xb[:, ib, :],
                        wT[ib][:, ob * NB : (ob + 1) * NB],
                        start=(ib == 0),
                        stop=(ib == IB - 1),
                    )

            def evac_store(tb, ps):
                yo = out_pool.tile(
                    [P, O_HALF], mybir.dt.float32, name=f"yo_{tb}", tag="yo"
                )
                for ob in range(OB):
                    nc.any.tensor_copy(out=yo[:, ob * NB : (ob + 1) * NB], in_=ps[ob][:])
                nc.sync.dma_start(y_ap[tb * P : (tb + 1) * P, :], yo[:])

            # ---- first two token blocks: K-major interleaved pair ----
            npair = min(2, tb_count)
            xbs = [load_cast(tb) for tb in range(npair)]
            pss = [alloc_psum(tb) for tb in range(npair)]
            for ib in range(IB):
                for t in range(npair):
                    mm_group(pss[t], xbs[t], ib)
            for t in range(npair):
                evac_store(t, pss[t])

            # ---- steady state: one token block at a time ----
            for tb in range(npair, tb_count):
                xb = load_cast(tb)
                ps = alloc_psum(tb)
                for ib in range(IB):
                    mm_group(ps, xb, ib)
                evac_store(tb, ps)

    nc.compile()
    _NC_CACHE[tb_count] = nc
    return nc


def _shard_inputs(x, W):
    import ml_dtypes

    x = np.asarray(x)
    if x.dtype != np.float32:
        x = x.astype(np.float32)
    W = np.asarray(W)
    if W.dtype != ml_dtypes.bfloat16:
        W = W.astype(ml_dtypes.bfloat16)
    tb_count = GROUP // P
    in_maps = []
    for c in range(N_CORES):
        g, h = c // 2, c % 2
        xg = x[g * GROUP : (g + 1) * GROUP]
        # t-block-tiled transpose: [TB, HIDDEN, 128], element (tb, i, t) =
        # x[g*GROUP + tb*128 + t, i]  (layout-only; values unchanged)
        xt = np.ascontiguousarray(xg.reshape(tb_count, P, HIDDEN).transpose(0, 2, 1))
        in_maps.append(
            {
                "x": xt,
                # weight shard shipped transposed: [HIDDEN, O_HALF]
                "w": np.ascontiguousarray(W[g, h * O_HALF : (h + 1) * O_HALF, :].T),
            }
        )
    return in_maps


def kernel(x, W, group_sizes=None, **_ignored):
    if group_sizes is not None:
        gs = np.asarray(group_sizes).astype(np.int64)
        assert gs.shape == (NUM_EXPERTS,) and np.all(gs == GROUP), (
            f"kernel compiled for static group_sizes=[{GROUP}]*{NUM_EXPERTS}, got {gs}"
        )
    _ensure_paths()
    from concourse.bass_utils import run_bass_kernel_spmd

    nc = build_nc()
    in_maps = _shard_inputs(x, W)
    res = run_bass_kernel_spmd(nc, in_maps, core_ids=list(range(N_CORES)))
    y = np.empty((TOTAL, HIDDEN), dtype=np.float32)
    for c in range(N_CORES):
        g, h = c // 2, c % 2
        y[g * GROUP : (g + 1) * GROUP, h * O_HALF : (h + 1) * O_HALF] = res.results[c][
            "y"
        ]
    return y
